# revision 5
# baseline (speedup 1.0000x reference)
"""Trainium2 Bass kernel for nn_Actor (GNN message passing + Beta policy head).

Strategy (data-parallel over graphs, 8 NeuronCores, no collectives):
  - Each core owns 32 graphs (2048 nodes, 65536 edges).
  - EdgeConv algebra: agg = (sum_k relu(h_k)) @ W2 + DEG*b2, and the head only
    needs 6 scalar projections of agg per node, so W2 and the head weights are
    composed host-side into C_j = W2 @ p_j and the per-edge work is only the
    first-layer matmul + relu + k-sum.
  - First layer per edge, hid-major (hid half on PSUM partitions), all
    matmuls fp8e4 DoubleRow with 128-partition stationaries (same tile size
    everywhere - mixed tile sizes serialize the PE):
      FAST mode (default): ONE pass per (k, half): cells = [ea(16 pair
        cells); xj(32); u-injection one-hots(64); zero(16)], where the
        per-node term u = x@W1a + b1 rides in the stream as fp8
        error-diffusion versions cycled k%4 (residual decorrelates over the
        32-edge sum). 324 ns/k/half/chunk steady state.
      SAFE mode: A pass = [ea; xj; zeros], plus a second identity-pair pass
        injecting u as fp8 hi+lo (exact to ~13 bits). 540 ns/k.
  - relu on ACT/DVE alternating, fp8e4 pair output; k-sum via identity-pair
    DoubleRow matmuls (2 k per pass) accumulating in PSUM.
  - Projections P = C^T @ rT (fp16), head combine via ring-shifted DVE adds,
    softplus ratio in log-space on [32, 136].
"""
import sys
import numpy as np

for _p in ("/opt/trn_rl_repo", "/opt/pypackages"):
    if _p not in sys.path:
        sys.path.insert(0, _p)

import ml_dtypes

B, NN, NODE, EDGEF, HID, DEG, NFACT = 256, 64, 64, 32, 256, 32, 8
NCORES = 8
BL = B // NCORES            # 32 graphs / core
NL = BL * NN                # 2048 nodes / core
EL = NL * DEG               # 65536 edges / core
NE_HEAD = 128
NCH = 4                     # node chunks of 512
CW = NL // NCH              # 512 nodes per chunk
NVER = 4                    # u error-diffusion versions (FAST mode)
E4NP = ml_dtypes.float8_e4m3

FAST = True

_COMPILED = {}


# --------------------------------------------------------------------------
# Device program
# --------------------------------------------------------------------------
def _build_nc(fast):
    import concourse.bass as bass
    import concourse.mybir as mybir
    from concourse import bacc, tile

    f16 = mybir.dt.float16
    f32 = mybir.dt.float32
    f8 = mybir.dt.float8e4
    AF = mybir.ActivationFunctionType
    DR = mybir.MatmulPerfMode.DoubleRow

    nc = bacc.Bacc("TRN2", target_bir_lowering=False, debug=False)

    # ---- DRAM inputs ----
    # FAST: stream is per (half, chunk, k): [128, 2, 512]
    # SAFE: stream is per (chunk, k) (halves share): [128, 2, 512]
    nstream = (2 if fast else 1) * NCH * DEG * 2 * CW
    SA_d = nc.dram_tensor("sa8", [128, nstream], f8, kind="ExternalInput")
    if not fast:
        UHL_d = nc.dram_tensor("uhl8", [128, 2 * NCH * 2 * CW], f8,
                               kind="ExternalInput")
    WA_d = nc.dram_tensor("wa8", [128, 2 * 256], f8, kind="ExternalInput")
    IDD_d = nc.dram_tensor("idd8", [128, 256], f8, kind="ExternalInput")
    CT_d = nc.dram_tensor("ct16", [128, 16], f16, kind="ExternalInput")
    AXA_d = nc.dram_tensor("axpa", [4, NL], f32, kind="ExternalInput")
    AXB_d = nc.dram_tensor("axpb", [2, NL], f32, kind="ExternalInput")
    HIGH_d = nc.dram_tensor("high_b", [BL, 136], f32, kind="ExternalInput")
    OUT_d = nc.dram_tensor("out", [BL, 136], f32, kind="ExternalOutput")

    with tile.TileContext(nc) as tc:
        with tc.tile_pool(name="const", bufs=1) as cpool:
            WA = cpool.tile([128, 2 * 256], f8, name="WA")
            nc.scalar.dma_start(WA[:, :], WA_d[:, :])
            IDD = cpool.tile([128, 256], f8, name="IDD")
            nc.scalar.dma_start(IDD[:, :], IDD_d[:, :])
            CT = cpool.tile([128, 16], f16, name="CT")
            if not fast:
                UHL = cpool.tile([128, 2 * NCH * 2 * CW], f8, name="UHL")
                nc.sync.dma_start(UHL[:, :], UHL_d[:, :])
                UHL_r = UHL[:, :].rearrange("p (h c g n) -> p h c g n",
                                            h=2, c=NCH, g=2)
            AXA = cpool.tile([4, NL], f32, name="AXA")
            AXB = cpool.tile([2, NL], f32, name="AXB")
            HIGH = cpool.tile([BL, 136], f32, name="HIGH")

            def load_late_consts():
                nc.scalar.dma_start(CT[:, :], CT_d[:, :])
                nc.scalar.dma_start(AXA[:, :], AXA_d[:, :])
                nc.scalar.dma_start(AXB[:, :], AXB_d[:, :])
                nc.scalar.dma_start(HIGH[:, :], HIGH_d[:, :])
            late_loaded = [False]

            rT = [cpool.tile([128, NL], f16, name=f"rT{h}") for h in range(2)]
            PJA = cpool.tile([4, NL], f32, name="PJA")
            PJB = cpool.tile([2, NL], f32, name="PJB")

            WA_r = WA[:, :].rearrange("p (h g m) -> p h g m", h=2, g=2)
            IDD_r = IDD[:, :].rearrange("p (g m) -> p g m", g=2)

            NSB = DEG // 2   # 16 superblocks of 2 k
            relu_ctr = [0]
            nhalf = 2 if fast else 1   # stream blocks per chunk

            with (
                tc.tile_pool(name="sas", bufs=3) as sapool,
                tc.tile_pool(name="rlp", bufs=2) as rlpool,
                tc.tile_pool(name="hps", bufs=3, space="PSUM") as hpool,
                tc.tile_pool(name="prs", bufs=2, space="PSUM") as prpool,
            ):
                KSZ = DEG * 2 * CW           # stream bytes per (half-)chunk
                ADLY = 3                     # accum delay in superblocks

                def emit_accum(st, psb):
                    PR, RL_r, half, ch = st
                    nc.tensor.matmul(
                        PR[:, :], IDD_r[0:128, :, 0:128],
                        RL_r[0:128, psb % 6, :, :],
                        start=(psb == 0), stop=(psb == NSB - 1),
                        perf_mode=DR)

                def emit_evac(st):
                    PR, RL_r, half, ch = st
                    nc.scalar.activation(
                        rT[half][:, ch * CW:(ch + 1) * CW], PR[:, :],
                        AF.Copy)

                def emit_proj(ch):
                    # PP: one PSUM bank; PPA rows at partitions 0..4,
                    # PPB rows at 32..34 (col-tile position 32)
                    PP = prpool.tile([128, CW], f32, name="PP", tag="pr")
                    for hh in range(2):
                        nc.tensor.matmul(
                            PP[0:4, :], CT[:, hh * 8:hh * 8 + 4],
                            rT[hh][:, ch * CW:(ch + 1) * CW],
                            start=(hh == 0), stop=(hh == 1))
                    for hh in range(2):
                        nc.tensor.matmul(
                            PP[32:34, :], CT[:, hh * 8 + 4:hh * 8 + 6],
                            rT[hh][:, ch * CW:(ch + 1) * CW],
                            start=(hh == 0), stop=(hh == 1))
                    nc.vector.tensor_add(
                        PJA[:, ch * CW:(ch + 1) * CW], PP[0:4, :],
                        AXA[:, ch * CW:(ch + 1) * CW])
                    nc.vector.tensor_add(
                        PJB[:, ch * CW:(ch + 1) * CW], PP[32:34, :],
                        AXB[:, ch * CW:(ch + 1) * CW])

                prev = None   # previous stage state
                sa_cache = {}
                for ch in range(NCH):
                    sa_tiles = []
                    for sh in range(nhalf):
                        SA = sapool.tile([128, KSZ], f8,
                                         name=f"SA{ch}_{sh}", tag="sa")
                        base = (ch * nhalf + sh) * KSZ
                        if ch == 0 and sh == 0:
                            kb_edges = [0, 1, 2, 4, 8, 16, 24, 32]
                        else:
                            kb_edges = [0, 8, 16, 24, 32]
                        for kb in range(len(kb_edges) - 1):
                            lo = kb_edges[kb] * 2 * CW
                            hi = kb_edges[kb + 1] * 2 * CW
                            nc.sync.dma_start(
                                SA[:, lo:hi], SA_d[:, base + lo:base + hi])
                        sa_tiles.append(SA[:, :].rearrange(
                            "p (k g n) -> p k g n", k=DEG, g=2))
                    if not late_loaded[0]:
                        load_late_consts()
                        late_loaded[0] = True
                    for half in range(2):
                        SA_r = sa_tiles[half if fast else 0]
                        PR = prpool.tile([128, CW], f32, name="PR", tag="pr")
                        RL = rlpool.tile([128, 6 * 2 * CW], f8,
                                         name="RL", tag="rl")
                        RL_r = RL[:, :].rearrange("p (s g n) -> p s g n",
                                                  s=6, g=2)
                        cur = (PR, RL_r, half, ch)
                        for sb in range(NSB):
                            # drain previous stage's tail accums
                            if prev is not None and sb < ADLY:
                                emit_accum(prev, NSB - ADLY + sb)
                                if sb == ADLY - 1:
                                    emit_evac(prev)
                                    if prev[2] == 1:
                                        emit_proj(prev[3])
                            if sb >= ADLY:
                                emit_accum(cur, sb - ADLY)
                            H = hpool.tile([128, 2 * CW], f32,
                                           name="H", tag="h")
                            for kk in range(2):
                                k = sb * 2 + kk
                                if fast:
                                    nc.tensor.matmul(
                                        H[:, kk * CW:(kk + 1) * CW],
                                        WA_r[0:128, half, :, :],
                                        SA_r[0:128, k, :, :],
                                        start=True, stop=True, perf_mode=DR)
                                else:
                                    nc.tensor.matmul(
                                        H[:, kk * CW:(kk + 1) * CW],
                                        WA_r[0:128, half, :, :],
                                        SA_r[0:128, k, :, :],
                                        start=True, stop=False, perf_mode=DR)
                                    nc.tensor.matmul(
                                        H[:, kk * CW:(kk + 1) * CW],
                                        IDD_r[0:128, :, 0:128],
                                        UHL_r[0:128, half, ch, :, :],
                                        start=False, stop=True, perf_mode=DR)
                            # relu of this superblock, strict alternation;
                            # last two sbs split across both engines to cut
                            # the latency the next stage's drain accums see
                            rs = (sb % 6) * 2 * CW
                            i = relu_ctr[0]
                            relu_ctr[0] += 1
                            if sb == NSB - 1:
                                nc.scalar.activation(
                                    RL[:, rs:rs + CW], H[:, 0:CW], AF.Relu)
                                nc.vector.tensor_scalar_max(
                                    RL[:, rs + CW:rs + 2 * CW],
                                    H[:, CW:2 * CW], 0.0)
                            elif i % 2 == 0:
                                nc.scalar.activation(
                                    RL[:, rs:rs + 2 * CW], H[:, :], AF.Relu)
                            else:
                                nc.vector.tensor_scalar_max(
                                    RL[:, rs:rs + 2 * CW], H[:, :], 0.0)
                        prev = cur
                for psb in range(NSB - ADLY, NSB):
                    emit_accum(prev, psb)
                emit_evac(prev)
                emit_proj(prev[3])

            # ---------------- head combine ----------------
            # PJA rows: 0 mu1, 1 sig1, 2 fmu, 3 fsig; PJB rows: 0 mu2, 1 sig2
            GS = 66
            SDB = cpool.tile([2, BL * GS], f32, name="SDB")
            SDB_r = SDB[:, :].rearrange("p (g i) -> p g i", i=GS)
            PJB_r = PJB[:, :].rearrange("p (g i) -> p g i", i=NN)
            PJA_r = PJA[:, :].rearrange("p (g i) -> p g i", i=NN)
            nc.scalar.activation(SDB_r[:, :, 0:NN], PJB_r[:, :, :], AF.Copy)
            nc.scalar.activation(SDB_r[:, :, NN:GS], PJB_r[:, :, 0:GS - NN],
                                 AF.Copy)

            HEAD_SHIFTS = (1, 2)
            LA = [cpool.tile([2, NL], f32, name=f"LA{b}") for b in range(2)]
            for b, sh2 in enumerate(HEAD_SHIFTS):
                eng = nc.gpsimd if b == 0 else nc.vector
                eng.tensor_add(
                    LA[b][0:2, :].rearrange("p (g i) -> p g i", i=NN),
                    PJA_r[0:2, :, 0:NN], SDB_r[0:2, :, sh2:sh2 + NN])

            # -------- assemble logits [32, 136] + log-space softplus ratio --
            import concourse.mybir as mb
            Fa = cpool.tile([BL, 136], f32, name="Fa")
            Fb = cpool.tile([BL, 136], f32, name="Fb")
            for row, Ft in ((0, Fa), (1, Fb)):
                deng = nc.sync if row == 0 else nc.scalar
                for blk in range(2):
                    deng.dma_start(
                        Ft[:, blk * 64:(blk + 1) * 64],
                        LA[blk][row:row + 1, :].rearrange(
                            "p (g e) -> p g e", g=BL))
                deng.dma_start(
                    Ft[:, 128:136], PJA_r[2 + row:3 + row, :, 56:64])
            C10 = cpool.tile([BL, 1], f32, name="C10")
            nc.vector.memset(C10[:, :], 10.0)
            Cm10 = cpool.tile([BL, 1], f32, name="Cm10")
            nc.vector.memset(Cm10[:, :], -10.0)
            LN = []
            tiles = {}
            for row, Ft in ((0, Fa), (1, Fb)):
                Lhi = cpool.tile([BL, 136], f32, name=f"Lhi{row}")
                nc.vector.tensor_scalar_max(Lhi[:, :], Ft[:, :], 1.0)
                Lmid = cpool.tile([BL, 136], f32, name=f"Lmid{row}")
                nc.vector.tensor_scalar(Lmid[:, :], Ft[:, :], 15.0, -15.0,
                                        mb.AluOpType.min, mb.AluOpType.max)
                Mhi = cpool.tile([BL, 136], f32, name=f"Mhi{row}")
                Mlo = cpool.tile([BL, 136], f32, name=f"Mlo{row}")
                tiles[row] = (Ft, Lhi, Lmid, Mhi, Mlo)
            for row in (0, 1):
                Ft, Lhi, Lmid, Mhi, Mlo = tiles[row]
                nc.scalar.activation(Lmid[:, :], Lmid[:, :], AF.Exp)
                nc.scalar.activation(Mhi[:, :], Ft[:, :], AF.Sign,
                                     bias=Cm10[:, 0:1])
                nc.scalar.activation(Mlo[:, :], Ft[:, :], AF.Sign,
                                     bias=C10[:, 0:1])
            for row in (0, 1):
                Ft, Lhi, Lmid, Mhi, Mlo = tiles[row]
                nc.scalar.activation(Lhi[:, :], Lhi[:, :], AF.Ln)
                nc.scalar.activation(Lmid[:, :], Lmid[:, :], AF.Ln, bias=1.0)
                nc.scalar.activation(Lmid[:, :], Lmid[:, :], AF.Ln)
            for row in (0, 1):
                Ft, Lhi, Lmid, Mhi, Mlo = tiles[row]
                eng = nc.vector if row == 0 else nc.gpsimd
                eng.tensor_scalar(Mhi[:, :], Mhi[:, :], 1.0, 0.5,
                                  mb.AluOpType.add, mb.AluOpType.mult)
                eng.tensor_scalar(Mlo[:, :], Mlo[:, :], -0.5, 0.5,
                                  mb.AluOpType.mult, mb.AluOpType.add)
                Mmid = cpool.tile([BL, 136], f32, name=f"Mmid{row}")
                eng.tensor_add(Mmid[:, :], Mhi[:, :], Mlo[:, :])
                eng.tensor_scalar(Mmid[:, :], Mmid[:, :], -1.0, 1.0,
                                  mb.AluOpType.mult, mb.AluOpType.add)
                Ls = cpool.tile([BL, 136], f32, name=f"Ls{row}")
                eng.tensor_mul(Ls[:, :], Ft[:, :], Mlo[:, :])
                eng.tensor_mul(Lhi[:, :], Lhi[:, :], Mhi[:, :])
                eng.tensor_add(Ls[:, :], Ls[:, :], Lhi[:, :])
                eng.tensor_mul(Lmid[:, :], Lmid[:, :], Mmid[:, :])
                eng.tensor_add(Ls[:, :], Ls[:, :], Lmid[:, :])
                LN.append(Ls)
            DD = cpool.tile([BL, 136], f32, name="DD")
            nc.vector.tensor_sub(DD[:, :], LN[1][:, :], LN[0][:, :])
            nc.scalar.activation(DD[:, :], DD[:, :], AF.Exp)
            nc.vector.tensor_scalar_add(DD[:, :], DD[:, :], 1.0)
            RR = cpool.tile([BL, 136], f32, name="RR")
            nc.vector.reciprocal(RR[:, :], DD[:, :])
            OO = cpool.tile([BL, 136], f32, name="OO")
            OO = cpool.tile([BL, 136], f32, name="OO")
            nc.vector.tensor_mul(OO[:, :], RR[:, :], HIGH[:, :])
            nc.sync.dma_start(OUT_d[:, :], OO[:, :])

    nc.compile()
    return nc


# --------------------------------------------------------------------------
# Structure check + fallback
# --------------------------------------------------------------------------
def _check_structure(edge_index, edges):
    src, dst = np.asarray(edge_index[0]), np.asarray(edge_index[1])
    n_total = B * NN
    if src.shape[0] != n_total * DEG:
        return None
    if not np.array_equal(src, np.repeat(np.arange(n_total, dtype=src.dtype), DEG)):
        return None
    g_of = src // NN
    i_of = src % NN
    j_of = dst - g_of * NN
    if j_of.min() < 0 or j_of.max() >= NN:
        return None
    s_all = ((j_of - i_of) % NN).reshape(n_total, DEG)
    shifts = s_all[0]
    if not np.all(s_all == shifts[None, :]):
        return None
    edges = np.asarray(edges)
    if edges.shape != (NE_HEAD, 2):
        return None
    he_src = edges[:, 0].reshape(2, 64)
    if not np.array_equal(he_src, np.tile(np.arange(64), (2, 1))):
        return None
    hsh = ((edges[:, 1] - edges[:, 0]) % NN).reshape(2, 64)
    if not np.all(hsh == hsh[:, :1]):
        return None
    if (int(hsh[0, 0]), int(hsh[1, 0])) != (1, 2):
        return None
    return shifts


def _np_fallback(x, edge_index, edge_attr, edges, high,
                 W1, b1, W2, b2, Wmu, bmu, Wsig, bsig, Wmu2, bmu2, Wsig2, bsig2):
    xi = x[edge_index[0]]
    xj = x[edge_index[1]]
    msg_in = np.concatenate([xi, xj, edge_attr], axis=1)
    msg = np.maximum(msg_in @ W1 + b1, 0) @ W2 + b2
    agg = np.zeros((x.shape[0], HID), np.float32)
    np.add.at(agg, edge_index[0], msg)
    x_pp = np.concatenate([x, agg], axis=1).reshape(-1, NN, NODE + HID)
    ef = np.concatenate([x_pp[:, edges[:, 0], :], x_pp[:, edges[:, 1], :]], axis=2)

    def softplus(v):
        return np.log1p(np.exp(-np.abs(v))) + np.maximum(v, 0)
    alpha = softplus((ef @ Wmu).squeeze(-1) + bmu + 1e-20)
    beta = softplus((ef @ Wsig).squeeze(-1) + bsig + 1e-20)
    fact = x_pp[:, -NFACT:, :]
    alpha2 = softplus((fact @ Wmu2).squeeze(-1) + bmu2 + 1e-20)
    beta2 = softplus((fact @ Wsig2).squeeze(-1) + bsig2 + 1e-20)
    dis = (alpha + 1e-20) / (alpha + beta + 2e-20) * high[:-NFACT]
    ordr = (alpha2 + 1e-20) / (alpha2 + beta2 + 2e-20) * high[-NFACT:]
    return np.concatenate([dis, ordr], axis=-1).astype(np.float32)


# --------------------------------------------------------------------------
# Host-side preparation
# --------------------------------------------------------------------------
def _q8(a):
    return np.asarray(a, np.float32).astype(E4NP)


def _prep_in_maps(x, edge_attr, shifts, high,
                  W1, b1, W2, b2, Wmu, bmu, Wsig, bsig, Wmu2, bmu2, Wsig2, bsig2,
                  fast=None):
    if fast is None:
        fast = FAST
    W1a, W1b, W1c = W1[0:64], W1[64:128], W1[128:160]

    # WA stationary: [128, half, g, 128]
    WA = np.zeros((128, 2, 2, 128), np.float32)
    for h in range(2):
        WA[0:16, h, 0, :] = W1c[0::2, h * 128:(h + 1) * 128]
        WA[0:16, h, 1, :] = W1c[1::2, h * 128:(h + 1) * 128]
        WA[16:48, h, 0, :] = W1b[0::2, h * 128:(h + 1) * 128]
        WA[16:48, h, 1, :] = W1b[1::2, h * 128:(h + 1) * 128]
    if fast:
        # cells 48..112: one-hot u-injection: cell c, slot g -> hid row
        # 2*(c-48)+g of this half
        for c in range(48, 112):
            WA[c, :, 0, 2 * (c - 48)] = 1.0
            WA[c, :, 1, 2 * (c - 48) + 1] = 1.0
    WA8 = _q8(WA.reshape(128, 2 * 2 * 128))

    IDD = np.zeros((128, 2, 128), np.float32)
    idx = np.arange(128)
    IDD[idx, 0, idx] = 1.0
    IDD[idx, 1, idx] = 1.0
    IDD8 = _q8(IDD.reshape(128, 256))

    # projection weights: order mu1, sig1, fmu, fsig, mu2, sig2
    pvecs = [Wmu[64:320, 0], Wsig[64:320, 0], Wmu2[64:320, 0],
             Wsig2[64:320, 0], Wmu[384:640, 0], Wsig[384:640, 0]]
    CT = np.zeros((128, 2, 8), np.float32)
    consts = np.zeros(8, np.float32)
    for j, p in enumerate(pvecs):
        Cj = (W2.astype(np.float64) @ p.astype(np.float64)).astype(np.float32)
        CT[:, 0, j] = Cj[0:128]
        CT[:, 1, j] = Cj[128:256]
        consts[j] = DEG * float(b2.astype(np.float64) @ p.astype(np.float64))
    CT16 = np.ascontiguousarray(CT.reshape(128, 16)).astype(np.float16)

    # host x-projections + biases (same row order as pvecs)
    xvecs = [Wmu[0:64, 0], Wsig[0:64, 0], Wmu2[0:64, 0], Wsig2[0:64, 0],
             Wmu[320:384, 0], Wsig[320:384, 0]]
    biases = [bmu[0], bsig[0], bmu2[0], bsig2[0], 0.0, 0.0]

    HIGH = np.ascontiguousarray(np.tile(high[None, :], (BL, 1))).astype(np.float32)

    i_idx = np.arange(NN)
    in_maps = []
    for core in range(NCORES):
        xs = x[core * NL:(core + 1) * NL]                    # [2048, 64]
        eas = edge_attr[core * EL:(core + 1) * EL]           # [65536, 32]
        ea_r = eas.reshape(NL, DEG, EDGEF)
        xs3 = xs.reshape(BL, NN, NODE)

        # common data cells [48, NCH, DEG, 2, CW]
        DAT = np.zeros((48, NCH, DEG, 2, CW), np.float32)
        ea_t = ea_r.reshape(NCH, CW, DEG, EDGEF).transpose(3, 0, 2, 1)
        DAT[0:16, :, :, 0, :] = ea_t[0::2]
        DAT[0:16, :, :, 1, :] = ea_t[1::2]
        xjv = np.empty((NODE, NCH, DEG, CW), np.float32)
        for k in range(DEG):
            xk = xs3[:, (i_idx + int(shifts[k])) % NN, :].reshape(NL, NODE)
            xjv[:, :, k, :] = xk.T.reshape(NODE, NCH, CW)
        DAT[16:48, :, :, 0, :] = xjv[0::2]
        DAT[16:48, :, :, 1, :] = xjv[1::2]
        DAT8 = _q8(DAT)

        # u = x @ W1a + b1
        u = (xs.astype(np.float64) @ W1a.astype(np.float64)).astype(np.float32) \
            + b1.astype(np.float32)[None, :]                 # [NL, 256]

        if fast:
            # error-diffusion versions v_j: sum_j v_j tracks j*u
            uvs = []
            acc = np.zeros_like(u)
            for j in range(NVER):
                v = _q8(u * (j + 1) - acc)
                uvs.append(v)
                acc = acc + v.astype(np.float32)
            # SA8: [128, half, NCH, DEG, 2, CW]
            SA = np.zeros((128, 2, NCH, DEG, 2, CW), E4NP)
            SA[0:48, 0] = DAT8
            SA[0:48, 1] = DAT8
            # cells 48..112: u-version pairs: cell c slot g holds hid row
            # 2*(c-48)+g of half, nodes of chunk, version k % NVER
            for half in range(2):
                for v in range(NVER):
                    uvT = uvs[v].T                            # [256, NL]
                    rows = uvT[half * 128:(half + 1) * 128]   # [128, NL]
                    rc = rows.reshape(64, 2, NCH, CW)         # [cell,g,ch,n]
                    for k in range(v, DEG, NVER):
                        SA[48:112, half, :, k, 0, :] = rc[:, 0, :, :]
                        SA[48:112, half, :, k, 1, :] = rc[:, 1, :, :]
            # device stream order is (chunk, half, k, g, n)
            SA8 = np.ascontiguousarray(
                SA.transpose(0, 2, 1, 3, 4, 5).reshape(
                    128, NCH * 2 * DEG * 2 * CW))
            m = {"sa8": SA8}
        else:
            SA = np.zeros((128, NCH, DEG, 2, CW), E4NP)
            SA[0:48] = DAT8
            SA8 = np.ascontiguousarray(SA.reshape(128, NCH * DEG * 2 * CW))
            u_hi = _q8(u)
            u_lo = _q8(u - u_hi.astype(np.float32))
            UHL = np.empty((128, 2, NCH, 2, CW), E4NP)
            uT_hi = u_hi.T.reshape(2, 128, NCH, CW)
            uT_lo = u_lo.T.reshape(2, 128, NCH, CW)
            UHL[:, 0, :, 0, :] = uT_hi[0]
            UHL[:, 0, :, 1, :] = uT_lo[0]
            UHL[:, 1, :, 0, :] = uT_hi[1]
            UHL[:, 1, :, 1, :] = uT_lo[1]
            m = {"sa8": SA8,
                 "uhl8": np.ascontiguousarray(UHL.reshape(128, 2 * NCH * 2 * CW))}

        AXm = np.zeros((6, NL), np.float32)
        for j in range(6):
            AXm[j] = xs @ xvecs[j] + biases[j] + consts[j]

        m.update({
            "wa8": WA8, "idd8": IDD8, "ct16": CT16,
            "axpa": np.ascontiguousarray(AXm[0:4]),
            "axpb": np.ascontiguousarray(AXm[4:6]), "high_b": HIGH,
        })
        in_maps.append(m)
    return in_maps


def kernel(**inputs):
    x = np.asarray(inputs["x"], np.float32)
    edge_index = np.asarray(inputs["edge_index"])
    edge_attr = np.asarray(inputs["edge_attr"], np.float32)
    edges = np.asarray(inputs["edges"])
    high = np.asarray(inputs["high"], np.float32)
    names = ["W1", "b1", "W2", "b2", "Wmu", "bmu", "Wsig", "bsig",
             "Wmu2", "bmu2", "Wsig2", "bsig2"]
    ws = {n: np.asarray(inputs[n], np.float32) for n in names}

    shifts = _check_structure(edge_index, edges)
    if shifts is None:
        return _np_fallback(x, edge_index, edge_attr, edges, high,
                            *[ws[n] for n in names])

    in_maps = _prep_in_maps(x, edge_attr, shifts, high,
                            *[ws[n] for n in names])

    key = f"nc_f{int(FAST)}"
    if key not in _COMPILED:
        _COMPILED[key] = _build_nc(FAST)
    nc = _COMPILED[key]

    from concourse.bass_utils import run_bass_kernel_spmd
    res = run_bass_kernel_spmd(nc, in_maps, core_ids=list(range(NCORES)))
    out = np.concatenate([res.results[c]["out"] for c in range(NCORES)], axis=0)
    return out.astype(np.float32)


if __name__ == "__main__":
    print("building nc...")
    nc = _build_nc(FAST)
    print("compiled OK")


# revision 7
# speedup vs baseline: 1.0109x; 1.0109x over previous
"""Trainium2 Bass kernel for nn_Actor (GNN message passing + Beta policy head).

Strategy (data-parallel over graphs, 8 NeuronCores, no collectives):
  - Each core owns 32 graphs (2048 nodes, 65536 edges).
  - EdgeConv algebra: agg = (sum_k relu(h_k)) @ W2 + DEG*b2, and the head only
    needs 6 scalar projections of agg per node, so W2 and the head weights are
    composed host-side into C_j = W2 @ p_j and the per-edge work is only the
    first-layer matmul + relu + k-sum.
  - First layer per edge, hid-major (hid half on PSUM partitions), all
    matmuls fp8e4 DoubleRow with 128-partition stationaries (same tile size
    everywhere - mixed tile sizes serialize the PE):
      FAST mode (default): ONE pass per (k, half): cells = [ea(16 pair
        cells); xj(32); u-injection one-hots(64); zero(16)], where the
        per-node term u = x@W1a + b1 rides in the stream as fp8
        error-diffusion versions cycled k%4 (residual decorrelates over the
        32-edge sum). 324 ns/k/half/chunk steady state.
      SAFE mode: A pass = [ea; xj; zeros], plus a second identity-pair pass
        injecting u as fp8 hi+lo (exact to ~13 bits). 540 ns/k.
  - relu on ACT/DVE alternating, fp8e4 pair output; k-sum via identity-pair
    DoubleRow matmuls (2 k per pass) accumulating in PSUM.
  - Projections P = C^T @ rT (fp16), head combine via ring-shifted DVE adds,
    softplus ratio in log-space on [32, 136].
"""
import sys
import numpy as np

for _p in ("/opt/trn_rl_repo", "/opt/pypackages"):
    if _p not in sys.path:
        sys.path.insert(0, _p)

import ml_dtypes

B, NN, NODE, EDGEF, HID, DEG, NFACT = 256, 64, 64, 32, 256, 32, 8
NCORES = 8
BL = B // NCORES            # 32 graphs / core
NL = BL * NN                # 2048 nodes / core
EL = NL * DEG               # 65536 edges / core
NE_HEAD = 128
NCH = 4                     # node chunks of 512
CW = NL // NCH              # 512 nodes per chunk
NVER = 4                    # u error-diffusion versions (FAST mode)
E4NP = ml_dtypes.float8_e4m3

FAST = True

_COMPILED = {}


# --------------------------------------------------------------------------
# Device program
# --------------------------------------------------------------------------
def _build_nc(fast):
    import concourse.bass as bass
    import concourse.mybir as mybir
    from concourse import bacc, tile

    f16 = mybir.dt.float16
    f32 = mybir.dt.float32
    f8 = mybir.dt.float8e4
    AF = mybir.ActivationFunctionType
    DR = mybir.MatmulPerfMode.DoubleRow

    nc = bacc.Bacc("TRN2", target_bir_lowering=False, debug=False)

    # ---- DRAM inputs ----
    # FAST: stream is per (half, chunk, k): [128, 2, 512]
    # SAFE: stream is per (chunk, k) (halves share): [128, 2, 512]
    nstream = (2 if fast else 1) * NCH * DEG * 2 * CW
    SA_d = nc.dram_tensor("sa8", [128, nstream], f8, kind="ExternalInput")
    if not fast:
        UHL_d = nc.dram_tensor("uhl8", [128, 2 * NCH * 2 * CW], f8,
                               kind="ExternalInput")
    WA_d = nc.dram_tensor("wa8", [128, 2 * 256], f8, kind="ExternalInput")
    IDD_d = nc.dram_tensor("idd8", [128, 256], f8, kind="ExternalInput")
    CT_d = nc.dram_tensor("ct16", [128, 16], f16, kind="ExternalInput")
    AXA_d = nc.dram_tensor("axpa", [4, NL], f32, kind="ExternalInput")
    AXB_d = nc.dram_tensor("axpb", [2, NL], f32, kind="ExternalInput")
    HIGH_d = nc.dram_tensor("high_b", [BL, 136], f32, kind="ExternalInput")
    OUT_d = nc.dram_tensor("out", [BL, 136], f32, kind="ExternalOutput")

    with tile.TileContext(nc) as tc:
        with tc.tile_pool(name="const", bufs=1) as cpool:
            WA = cpool.tile([128, 2 * 256], f8, name="WA")
            nc.scalar.dma_start(WA[:, :], WA_d[:, :])
            IDD = cpool.tile([128, 256], f8, name="IDD")
            nc.scalar.dma_start(IDD[:, :], IDD_d[:, :])
            CT = cpool.tile([128, 16], f16, name="CT")
            if not fast:
                UHL = cpool.tile([128, 2 * NCH * 2 * CW], f8, name="UHL")
                nc.sync.dma_start(UHL[:, :], UHL_d[:, :])
                UHL_r = UHL[:, :].rearrange("p (h c g n) -> p h c g n",
                                            h=2, c=NCH, g=2)
            AXA = cpool.tile([4, NL], f32, name="AXA")
            AXB = cpool.tile([2, NL], f32, name="AXB")
            HIGH = cpool.tile([BL, 136], f32, name="HIGH")

            def load_late_consts():
                nc.scalar.dma_start(CT[:, :], CT_d[:, :])
                nc.scalar.dma_start(AXA[:, :], AXA_d[:, :])
                nc.scalar.dma_start(AXB[:, :], AXB_d[:, :])
                nc.scalar.dma_start(HIGH[:, :], HIGH_d[:, :])
            late_loaded = [False]

            rT = [cpool.tile([128, NL], f16, name=f"rT{h}") for h in range(2)]
            PJA = cpool.tile([4, NL], f32, name="PJA")
            PJB = cpool.tile([2, NL], f32, name="PJB")
            GS = 66
            SDB = cpool.tile([2, BL * GS], f32, name="SDB")
            SDB_r = SDB[:, :].rearrange("p (g i) -> p g i", i=GS)
            PJB_r = PJB[:, :].rearrange("p (g i) -> p g i", i=NN)
            PJA_r = PJA[:, :].rearrange("p (g i) -> p g i", i=NN)
            HEAD_SHIFTS = (1, 2)
            LA = [cpool.tile([2, NL], f32, name=f"LA{b}") for b in range(2)]
            GPC = BL // NCH   # graphs per chunk

            WA_r = WA[:, :].rearrange("p (h g m) -> p h g m", h=2, g=2)
            IDD_r = IDD[:, :].rearrange("p (g m) -> p g m", g=2)

            NSB = DEG // 2   # 16 superblocks of 2 k
            relu_ctr = [0]
            nhalf = 2 if fast else 1   # stream blocks per chunk

            with (
                tc.tile_pool(name="sas", bufs=3) as sapool,
                tc.tile_pool(name="rlp", bufs=2) as rlpool,
                tc.tile_pool(name="hps", bufs=3, space="PSUM") as hpool,
                tc.tile_pool(name="prs", bufs=2, space="PSUM") as prpool,
            ):
                KSZ = DEG * 2 * CW           # stream bytes per (half-)chunk
                ADLY = 3                     # accum delay in superblocks

                def emit_accum(st, psb):
                    PR, RL_r, half, ch = st
                    nc.tensor.matmul(
                        PR[:, :], IDD_r[0:128, :, 0:128],
                        RL_r[0:128, psb % 6, :, :],
                        start=(psb == 0), stop=(psb == NSB - 1),
                        perf_mode=DR)

                def emit_evac(st):
                    PR, RL_r, half, ch = st
                    nc.scalar.activation(
                        rT[half][:, ch * CW:(ch + 1) * CW], PR[:, :],
                        AF.Copy)

                def emit_proj(ch):
                    # PP: one PSUM bank; PPA rows at partitions 0..4,
                    # PPB rows at 32..34 (col-tile position 32)
                    PP = prpool.tile([128, CW], f32, name="PP", tag="pr")
                    for hh in range(2):
                        nc.tensor.matmul(
                            PP[0:4, :], CT[:, hh * 8:hh * 8 + 4],
                            rT[hh][:, ch * CW:(ch + 1) * CW],
                            start=(hh == 0), stop=(hh == 1))
                    for hh in range(2):
                        nc.tensor.matmul(
                            PP[32:34, :], CT[:, hh * 8 + 4:hh * 8 + 6],
                            rT[hh][:, ch * CW:(ch + 1) * CW],
                            start=(hh == 0), stop=(hh == 1))
                    nc.vector.tensor_add(
                        PJA[:, ch * CW:(ch + 1) * CW], PP[0:4, :],
                        AXA[:, ch * CW:(ch + 1) * CW])
                    nc.vector.tensor_add(
                        PJB[:, ch * CW:(ch + 1) * CW], PP[32:34, :],
                        AXB[:, ch * CW:(ch + 1) * CW])

                prev = None   # previous stage state
                sa_cache = {}
                for ch in range(NCH):
                    sa_tiles = []
                    for sh in range(nhalf):
                        SA = sapool.tile([128, KSZ], f8,
                                         name=f"SA{ch}_{sh}", tag="sa")
                        base = (ch * nhalf + sh) * KSZ
                        if ch == 0 and sh == 0:
                            kb_edges = [0, 1, 2, 4, 8, 16, 24, 32]
                        else:
                            kb_edges = [0, 8, 16, 24, 32]
                        for kb in range(len(kb_edges) - 1):
                            lo = kb_edges[kb] * 2 * CW
                            hi = kb_edges[kb + 1] * 2 * CW
                            nc.sync.dma_start(
                                SA[:, lo:hi], SA_d[:, base + lo:base + hi])
                        sa_tiles.append(SA[:, :].rearrange(
                            "p (k g n) -> p k g n", k=DEG, g=2))
                    if not late_loaded[0]:
                        load_late_consts()
                        late_loaded[0] = True
                    for half in range(2):
                        SA_r = sa_tiles[half if fast else 0]
                        PR = prpool.tile([128, CW], f32, name="PR", tag="pr")
                        RL = rlpool.tile([128, 6 * 2 * CW], f8,
                                         name="RL", tag="rl")
                        RL_r = RL[:, :].rearrange("p (s g n) -> p s g n",
                                                  s=6, g=2)
                        cur = (PR, RL_r, half, ch)
                        for sb in range(NSB):
                            # drain previous stage's tail accums
                            if prev is not None and sb < ADLY:
                                emit_accum(prev, NSB - ADLY + sb)
                                if sb == ADLY - 1:
                                    emit_evac(prev)
                            if sb >= ADLY:
                                emit_accum(cur, sb - ADLY)
                            H = hpool.tile([128, 2 * CW], f32,
                                           name="H", tag="h")
                            for kk in range(2):
                                k = sb * 2 + kk
                                if fast:
                                    nc.tensor.matmul(
                                        H[:, kk * CW:(kk + 1) * CW],
                                        WA_r[0:128, half, :, :],
                                        SA_r[0:128, k, :, :],
                                        start=True, stop=True, perf_mode=DR)
                                else:
                                    nc.tensor.matmul(
                                        H[:, kk * CW:(kk + 1) * CW],
                                        WA_r[0:128, half, :, :],
                                        SA_r[0:128, k, :, :],
                                        start=True, stop=False, perf_mode=DR)
                                    nc.tensor.matmul(
                                        H[:, kk * CW:(kk + 1) * CW],
                                        IDD_r[0:128, :, 0:128],
                                        UHL_r[0:128, half, ch, :, :],
                                        start=False, stop=True, perf_mode=DR)
                            # relu of this superblock, strict alternation;
                            # last two sbs split across both engines to cut
                            # the latency the next stage's drain accums see
                            rs = (sb % 6) * 2 * CW
                            i = relu_ctr[0]
                            relu_ctr[0] += 1
                            if sb == NSB - 1:
                                nc.scalar.activation(
                                    RL[:, rs:rs + CW], H[:, 0:CW], AF.Relu)
                                nc.vector.tensor_scalar_max(
                                    RL[:, rs + CW:rs + 2 * CW],
                                    H[:, CW:2 * CW], 0.0)
                            elif i % 2 == 0:
                                nc.scalar.activation(
                                    RL[:, rs:rs + 2 * CW], H[:, :], AF.Relu)
                            else:
                                nc.vector.tensor_scalar_max(
                                    RL[:, rs:rs + 2 * CW], H[:, :], 0.0)
                        prev = cur
                for psb in range(NSB - ADLY, NSB):
                    emit_accum(prev, psb)
                emit_evac(prev)
                for pch in range(NCH):
                    emit_proj(pch)

            # ---------------- head combine ----------------
            nc.scalar.activation(SDB_r[:, :, 0:NN], PJB_r[:, :, :], AF.Copy)
            nc.scalar.activation(SDB_r[:, :, NN:GS], PJB_r[:, :, 0:GS - NN],
                                 AF.Copy)
            for b, sh2 in enumerate(HEAD_SHIFTS):
                nc.vector.tensor_add(
                    LA[b][0:2, :].rearrange("p (g i) -> p g i", i=NN),
                    PJA_r[0:2, :, 0:NN], SDB_r[0:2, :, sh2:sh2 + NN])

            # -------- assemble logits [32, 136] + log-space softplus ratio --
            import concourse.mybir as mb
            Fa = cpool.tile([BL, 136], f32, name="Fa")
            Fb = cpool.tile([BL, 136], f32, name="Fb")
            for row, Ft in ((0, Fa), (1, Fb)):
                deng = nc.sync if row == 0 else nc.scalar
                for blk in range(2):
                    deng.dma_start(
                        Ft[:, blk * 64:(blk + 1) * 64],
                        LA[blk][row:row + 1, :].rearrange(
                            "p (g e) -> p g e", g=BL))
                deng.dma_start(
                    Ft[:, 128:136], PJA_r[2 + row:3 + row, :, 56:64])
            C10 = cpool.tile([BL, 1], f32, name="C10")
            nc.vector.memset(C10[:, :], 10.0)
            Cm10 = cpool.tile([BL, 1], f32, name="Cm10")
            nc.vector.memset(Cm10[:, :], -10.0)
            LN = []
            tiles = {}
            for row, Ft in ((0, Fa), (1, Fb)):
                Lhi = cpool.tile([BL, 136], f32, name=f"Lhi{row}")
                nc.vector.tensor_scalar_max(Lhi[:, :], Ft[:, :], 1.0)
                Lmid = cpool.tile([BL, 136], f32, name=f"Lmid{row}")
                nc.vector.tensor_scalar(Lmid[:, :], Ft[:, :], 15.0, -15.0,
                                        mb.AluOpType.min, mb.AluOpType.max)
                Mhi = cpool.tile([BL, 136], f32, name=f"Mhi{row}")
                Mlo = cpool.tile([BL, 136], f32, name=f"Mlo{row}")
                tiles[row] = (Ft, Lhi, Lmid, Mhi, Mlo)
            for row in (0, 1):
                Ft, Lhi, Lmid, Mhi, Mlo = tiles[row]
                nc.scalar.activation(Lmid[:, :], Lmid[:, :], AF.Exp)
                nc.scalar.activation(Mhi[:, :], Ft[:, :], AF.Sign,
                                     bias=Cm10[:, 0:1])
                nc.scalar.activation(Mlo[:, :], Ft[:, :], AF.Sign,
                                     bias=C10[:, 0:1])
            for row in (0, 1):
                Ft, Lhi, Lmid, Mhi, Mlo = tiles[row]
                nc.scalar.activation(Lhi[:, :], Lhi[:, :], AF.Ln)
                nc.scalar.activation(Lmid[:, :], Lmid[:, :], AF.Ln, bias=1.0)
                nc.scalar.activation(Lmid[:, :], Lmid[:, :], AF.Ln)
            for row in (0, 1):
                Ft, Lhi, Lmid, Mhi, Mlo = tiles[row]
                eng = nc.vector if row == 0 else nc.gpsimd
                eng.tensor_scalar(Mhi[:, :], Mhi[:, :], 1.0, 0.5,
                                  mb.AluOpType.add, mb.AluOpType.mult)
                eng.tensor_scalar(Mlo[:, :], Mlo[:, :], -0.5, 0.5,
                                  mb.AluOpType.mult, mb.AluOpType.add)
                Mmid = cpool.tile([BL, 136], f32, name=f"Mmid{row}")
                eng.tensor_add(Mmid[:, :], Mhi[:, :], Mlo[:, :])
                eng.tensor_scalar(Mmid[:, :], Mmid[:, :], -1.0, 1.0,
                                  mb.AluOpType.mult, mb.AluOpType.add)
                Ls = cpool.tile([BL, 136], f32, name=f"Ls{row}")
                eng.tensor_mul(Ls[:, :], Ft[:, :], Mlo[:, :])
                eng.tensor_mul(Lhi[:, :], Lhi[:, :], Mhi[:, :])
                eng.tensor_add(Ls[:, :], Ls[:, :], Lhi[:, :])
                eng.tensor_mul(Lmid[:, :], Lmid[:, :], Mmid[:, :])
                eng.tensor_add(Ls[:, :], Ls[:, :], Lmid[:, :])
                LN.append(Ls)
            DD = cpool.tile([BL, 136], f32, name="DD")
            nc.vector.tensor_sub(DD[:, :], LN[1][:, :], LN[0][:, :])
            nc.scalar.activation(DD[:, :], DD[:, :], AF.Exp)
            nc.vector.tensor_scalar_add(DD[:, :], DD[:, :], 1.0)
            RR = cpool.tile([BL, 136], f32, name="RR")
            nc.vector.reciprocal(RR[:, :], DD[:, :])
            OO = cpool.tile([BL, 136], f32, name="OO")
            OO = cpool.tile([BL, 136], f32, name="OO")
            nc.vector.tensor_mul(OO[:, :], RR[:, :], HIGH[:, :])
            nc.sync.dma_start(OUT_d[:, :], OO[:, :])

    nc.compile()
    return nc


# --------------------------------------------------------------------------
# Structure check + fallback
# --------------------------------------------------------------------------
def _check_structure(edge_index, edges):
    src, dst = np.asarray(edge_index[0]), np.asarray(edge_index[1])
    n_total = B * NN
    if src.shape[0] != n_total * DEG:
        return None
    if not np.array_equal(src, np.repeat(np.arange(n_total, dtype=src.dtype), DEG)):
        return None
    g_of = src // NN
    i_of = src % NN
    j_of = dst - g_of * NN
    if j_of.min() < 0 or j_of.max() >= NN:
        return None
    s_all = ((j_of - i_of) % NN).reshape(n_total, DEG)
    shifts = s_all[0]
    if not np.all(s_all == shifts[None, :]):
        return None
    edges = np.asarray(edges)
    if edges.shape != (NE_HEAD, 2):
        return None
    he_src = edges[:, 0].reshape(2, 64)
    if not np.array_equal(he_src, np.tile(np.arange(64), (2, 1))):
        return None
    hsh = ((edges[:, 1] - edges[:, 0]) % NN).reshape(2, 64)
    if not np.all(hsh == hsh[:, :1]):
        return None
    if (int(hsh[0, 0]), int(hsh[1, 0])) != (1, 2):
        return None
    return shifts


def _np_fallback(x, edge_index, edge_attr, edges, high,
                 W1, b1, W2, b2, Wmu, bmu, Wsig, bsig, Wmu2, bmu2, Wsig2, bsig2):
    xi = x[edge_index[0]]
    xj = x[edge_index[1]]
    msg_in = np.concatenate([xi, xj, edge_attr], axis=1)
    msg = np.maximum(msg_in @ W1 + b1, 0) @ W2 + b2
    agg = np.zeros((x.shape[0], HID), np.float32)
    np.add.at(agg, edge_index[0], msg)
    x_pp = np.concatenate([x, agg], axis=1).reshape(-1, NN, NODE + HID)
    ef = np.concatenate([x_pp[:, edges[:, 0], :], x_pp[:, edges[:, 1], :]], axis=2)

    def softplus(v):
        return np.log1p(np.exp(-np.abs(v))) + np.maximum(v, 0)
    alpha = softplus((ef @ Wmu).squeeze(-1) + bmu + 1e-20)
    beta = softplus((ef @ Wsig).squeeze(-1) + bsig + 1e-20)
    fact = x_pp[:, -NFACT:, :]
    alpha2 = softplus((fact @ Wmu2).squeeze(-1) + bmu2 + 1e-20)
    beta2 = softplus((fact @ Wsig2).squeeze(-1) + bsig2 + 1e-20)
    dis = (alpha + 1e-20) / (alpha + beta + 2e-20) * high[:-NFACT]
    ordr = (alpha2 + 1e-20) / (alpha2 + beta2 + 2e-20) * high[-NFACT:]
    return np.concatenate([dis, ordr], axis=-1).astype(np.float32)


# --------------------------------------------------------------------------
# Host-side preparation
# --------------------------------------------------------------------------
def _q8(a):
    return np.asarray(a, np.float32).astype(E4NP)


def _prep_in_maps(x, edge_attr, shifts, high,
                  W1, b1, W2, b2, Wmu, bmu, Wsig, bsig, Wmu2, bmu2, Wsig2, bsig2,
                  fast=None):
    if fast is None:
        fast = FAST
    W1a, W1b, W1c = W1[0:64], W1[64:128], W1[128:160]

    # WA stationary: [128, half, g, 128]
    WA = np.zeros((128, 2, 2, 128), np.float32)
    for h in range(2):
        WA[0:16, h, 0, :] = W1c[0::2, h * 128:(h + 1) * 128]
        WA[0:16, h, 1, :] = W1c[1::2, h * 128:(h + 1) * 128]
        WA[16:48, h, 0, :] = W1b[0::2, h * 128:(h + 1) * 128]
        WA[16:48, h, 1, :] = W1b[1::2, h * 128:(h + 1) * 128]
    if fast:
        # cells 48..112: one-hot u-injection: cell c, slot g -> hid row
        # 2*(c-48)+g of this half
        for c in range(48, 112):
            WA[c, :, 0, 2 * (c - 48)] = 1.0
            WA[c, :, 1, 2 * (c - 48) + 1] = 1.0
    WA8 = _q8(WA.reshape(128, 2 * 2 * 128))

    IDD = np.zeros((128, 2, 128), np.float32)
    idx = np.arange(128)
    IDD[idx, 0, idx] = 1.0
    IDD[idx, 1, idx] = 1.0
    IDD8 = _q8(IDD.reshape(128, 256))

    # projection weights: order mu1, sig1, fmu, fsig, mu2, sig2
    pvecs = [Wmu[64:320, 0], Wsig[64:320, 0], Wmu2[64:320, 0],
             Wsig2[64:320, 0], Wmu[384:640, 0], Wsig[384:640, 0]]
    CT = np.zeros((128, 2, 8), np.float32)
    consts = np.zeros(8, np.float32)
    for j, p in enumerate(pvecs):
        Cj = (W2.astype(np.float64) @ p.astype(np.float64)).astype(np.float32)
        CT[:, 0, j] = Cj[0:128]
        CT[:, 1, j] = Cj[128:256]
        consts[j] = DEG * float(b2.astype(np.float64) @ p.astype(np.float64))
    CT16 = np.ascontiguousarray(CT.reshape(128, 16)).astype(np.float16)

    # host x-projections + biases (same row order as pvecs)
    xvecs = [Wmu[0:64, 0], Wsig[0:64, 0], Wmu2[0:64, 0], Wsig2[0:64, 0],
             Wmu[320:384, 0], Wsig[320:384, 0]]
    biases = [bmu[0], bsig[0], bmu2[0], bsig2[0], 0.0, 0.0]

    HIGH = np.ascontiguousarray(np.tile(high[None, :], (BL, 1))).astype(np.float32)

    i_idx = np.arange(NN)
    in_maps = []
    for core in range(NCORES):
        xs = x[core * NL:(core + 1) * NL]                    # [2048, 64]
        eas = edge_attr[core * EL:(core + 1) * EL]           # [65536, 32]
        ea_r = eas.reshape(NL, DEG, EDGEF)
        xs3 = xs.reshape(BL, NN, NODE)

        # common data cells [48, NCH, DEG, 2, CW]
        DAT = np.zeros((48, NCH, DEG, 2, CW), np.float32)
        ea_t = ea_r.reshape(NCH, CW, DEG, EDGEF).transpose(3, 0, 2, 1)
        DAT[0:16, :, :, 0, :] = ea_t[0::2]
        DAT[0:16, :, :, 1, :] = ea_t[1::2]
        xjv = np.empty((NODE, NCH, DEG, CW), np.float32)
        for k in range(DEG):
            xk = xs3[:, (i_idx + int(shifts[k])) % NN, :].reshape(NL, NODE)
            xjv[:, :, k, :] = xk.T.reshape(NODE, NCH, CW)
        DAT[16:48, :, :, 0, :] = xjv[0::2]
        DAT[16:48, :, :, 1, :] = xjv[1::2]
        DAT8 = _q8(DAT)

        # u = x @ W1a + b1
        u = (xs.astype(np.float64) @ W1a.astype(np.float64)).astype(np.float32) \
            + b1.astype(np.float32)[None, :]                 # [NL, 256]

        if fast:
            # error-diffusion versions v_j: sum_j v_j tracks j*u
            uvs = []
            acc = np.zeros_like(u)
            for j in range(NVER):
                v = _q8(u * (j + 1) - acc)
                uvs.append(v)
                acc = acc + v.astype(np.float32)
            # SA8: [128, half, NCH, DEG, 2, CW]
            SA = np.zeros((128, 2, NCH, DEG, 2, CW), E4NP)
            SA[0:48, 0] = DAT8
            SA[0:48, 1] = DAT8
            # cells 48..112: u-version pairs: cell c slot g holds hid row
            # 2*(c-48)+g of half, nodes of chunk, version k % NVER
            for half in range(2):
                for v in range(NVER):
                    uvT = uvs[v].T                            # [256, NL]
                    rows = uvT[half * 128:(half + 1) * 128]   # [128, NL]
                    rc = rows.reshape(64, 2, NCH, CW)         # [cell,g,ch,n]
                    for k in range(v, DEG, NVER):
                        SA[48:112, half, :, k, 0, :] = rc[:, 0, :, :]
                        SA[48:112, half, :, k, 1, :] = rc[:, 1, :, :]
            # device stream order is (chunk, half, k, g, n)
            SA8 = np.ascontiguousarray(
                SA.transpose(0, 2, 1, 3, 4, 5).reshape(
                    128, NCH * 2 * DEG * 2 * CW))
            m = {"sa8": SA8}
        else:
            SA = np.zeros((128, NCH, DEG, 2, CW), E4NP)
            SA[0:48] = DAT8
            SA8 = np.ascontiguousarray(SA.reshape(128, NCH * DEG * 2 * CW))
            u_hi = _q8(u)
            u_lo = _q8(u - u_hi.astype(np.float32))
            UHL = np.empty((128, 2, NCH, 2, CW), E4NP)
            uT_hi = u_hi.T.reshape(2, 128, NCH, CW)
            uT_lo = u_lo.T.reshape(2, 128, NCH, CW)
            UHL[:, 0, :, 0, :] = uT_hi[0]
            UHL[:, 0, :, 1, :] = uT_lo[0]
            UHL[:, 1, :, 0, :] = uT_hi[1]
            UHL[:, 1, :, 1, :] = uT_lo[1]
            m = {"sa8": SA8,
                 "uhl8": np.ascontiguousarray(UHL.reshape(128, 2 * NCH * 2 * CW))}

        AXm = np.zeros((6, NL), np.float32)
        for j in range(6):
            AXm[j] = xs @ xvecs[j] + biases[j] + consts[j]

        m.update({
            "wa8": WA8, "idd8": IDD8, "ct16": CT16,
            "axpa": np.ascontiguousarray(AXm[0:4]),
            "axpb": np.ascontiguousarray(AXm[4:6]), "high_b": HIGH,
        })
        in_maps.append(m)
    return in_maps


def kernel(**inputs):
    x = np.asarray(inputs["x"], np.float32)
    edge_index = np.asarray(inputs["edge_index"])
    edge_attr = np.asarray(inputs["edge_attr"], np.float32)
    edges = np.asarray(inputs["edges"])
    high = np.asarray(inputs["high"], np.float32)
    names = ["W1", "b1", "W2", "b2", "Wmu", "bmu", "Wsig", "bsig",
             "Wmu2", "bmu2", "Wsig2", "bsig2"]
    ws = {n: np.asarray(inputs[n], np.float32) for n in names}

    shifts = _check_structure(edge_index, edges)
    if shifts is None:
        return _np_fallback(x, edge_index, edge_attr, edges, high,
                            *[ws[n] for n in names])

    in_maps = _prep_in_maps(x, edge_attr, shifts, high,
                            *[ws[n] for n in names])

    key = f"nc_f{int(FAST)}"
    if key not in _COMPILED:
        _COMPILED[key] = _build_nc(FAST)
    nc = _COMPILED[key]

    from concourse.bass_utils import run_bass_kernel_spmd
    res = run_bass_kernel_spmd(nc, in_maps, core_ids=list(range(NCORES)))
    out = np.concatenate([res.results[c]["out"] for c in range(NCORES)], axis=0)
    return out.astype(np.float32)


if __name__ == "__main__":
    print("building nc...")
    nc = _build_nc(FAST)
    print("compiled OK")
